# revision 1
# baseline (speedup 1.0000x reference)
"""Trainium2 Bass kernel for nn_BareDotProdAttnEncoder (tree scan, gnn_message_passing).

Reference semantics (per batch element b):
  h_0 = x_0
  for i in 1..N-1:
      p = parent[i]  (p < i)
      alpha = exp(<h_p, x_i>); beta = exp(<x_i, x_i>)
      h_i = (alpha*h_p + beta*x_i) / (alpha + beta + 1e-15)

Equivalent form used on device:
  w = sigmoid(<h_p, x_i> - <x_i, x_i>)      (= alpha/(alpha+beta))
  h_i = w*(h_p - x_i) + x_i

Strategy: the recurrence only couples a node to its parent, and with
parent[i] < i drawn uniformly the trees are shallow (~18 levels for
N=2048). Process nodes level-by-level: all nodes of one level are
independent given the previous levels' h. All indices are known on the
host, so the host computes a level schedule and the device does batched
index gathers (SWDGE dma_gather), the dot/sigmoid/blend math, and
contiguous writebacks of each level into a persistent HBM state buffer
laid out level-contiguously ("sorted" order). The host inverse-permutes
the returned state into the reference node order.

Sharding: pure data parallelism over the batch; each of the 8 cores owns
4 trees, processed as 2 independent streams of 2 trees each (streams
pipeline against each other to hide per-level DMA latency).
"""

import os
import numpy as np

N_CORES = 8
STREAMS = int(os.environ.get("K_STREAMS", "2"))
TREES_PER_STREAM = 4 // STREAMS
DIM = 512
PART = 128
XBUFS = int(os.environ.get("K_XBUFS", "2"))
PBUFS = int(os.environ.get("K_PBUFS", "1"))
DBUFS = int(os.environ.get("K_DBUFS", "1"))
HBUFS = int(os.environ.get("K_HBUFS", "2"))
DSUB_ENG = os.environ.get("K_DSUB_ENG", "vector")
ABLATE = os.environ.get("K_ABLATE", "")
MAXLEV = int(os.environ.get("K_MAXLEV", "0"))  # 0 = all levels
SKIP = set(x for x in os.environ.get("K_SKIP", "").split(",") if x)
REPEAT = int(os.environ.get("K_REPEAT", "1"))
DYN = os.environ.get("K_DYN", "1") == "1"  # dynamic gather counts (skip pad traffic)
SINGLE_PACKET = os.environ.get("K_SINGLEPKT", "1") == "1"
WBSCATTER = os.environ.get("K_WBSCATTER", "0") == "1"  # exact-row wb via scatter-add


def _compute_depths(conn):
    B, N = conn.shape
    depths = np.zeros((B, N), np.int32)
    bidx = np.arange(B)
    for i in range(1, N):
        depths[:, i] = depths[bidx, conn[:, i]] + 1
    return depths


def _assign_trees(S, B):
    """Group trees into (stream, core) slots to minimize total padded chunks.
    S: per-tree level-size matrix [B, L]. Returns groups[g][c] = tuple of trees.
    Deterministic local search (seeded)."""
    L = S.shape[1]
    tps = TREES_PER_STREAM
    nslots = B // tps  # STREAMS * N_CORES
    nat = [tuple(range(tps * s, tps * (s + 1))) for s in range(nslots)]

    def cost(assign):
        tot = 0
        for g in range(STREAMS):
            lv = np.zeros(L, np.int64)
            for c in range(N_CORES):
                grp = assign[g * N_CORES + c]
                n = np.sum(S[list(grp)], axis=0)
                lv = np.maximum(lv, (n + PART - 1) // PART)
            tot += lv.sum()
        return int(tot)

    if os.environ.get("K_NATASSIGN", "0") == "1":
        return [[nat[g * N_CORES + c] for c in range(N_CORES)] for g in range(STREAMS)]
    rng = np.random.default_rng(12345)
    cur = [list(p) for p in nat]
    cc = cost([tuple(p) for p in cur])
    best, bc = [tuple(p) for p in cur], cc
    for _ in range(20000):
        a = int(rng.integers(0, nslots)); b2 = int(rng.integers(0, nslots))
        if a == b2:
            continue
        i = int(rng.integers(0, tps)); j = int(rng.integers(0, tps))
        cur[a][i], cur[b2][j] = cur[b2][j], cur[a][i]
        c2 = cost([tuple(p) for p in cur])
        if c2 <= cc:
            cc = c2
            if c2 < bc:
                best, bc = [tuple(p) for p in cur], c2
        else:
            cur[a][i], cur[b2][j] = cur[b2][j], cur[a][i]
    return [[best[g * N_CORES + c] for c in range(N_CORES)] for g in range(STREAMS)]


def _build_schedule(conn):
    """Host-side schedule: level structure, per-core index arrays, maps.

    Returns (L, Cls, sched) where
      L: number of levels
      Cls[g]: list of per-level chunk counts (uniform across cores)
      sched[c]: dict with per-core input arrays + posmat for assembly
    """
    B, N = conn.shape
    depths = _compute_depths(conn)
    L = int(depths.max()) + 1

    # node lists per (batch, level), ordered by node id (stable)
    order = [[np.nonzero(depths[b] == l)[0] for l in range(L)] for b in range(B)]

    S = np.zeros((B, L), np.int64)
    for b in range(B):
        S[b] = np.bincount(depths[b], minlength=L)
    groups = _assign_trees(S, B)  # groups[g][c] = tree tuple

    # uniform chunk capacities per stream
    Cls = []
    for g in range(STREAMS):
        Cl = np.zeros(L, np.int64)
        for c in range(N_CORES):
            trees = groups[g][c]
            for l in range(L):
                n = sum(len(order[b][l]) for b in trees)
                Cl[l] = max(Cl[l], (n + PART - 1) // PART)
        Cls.append([int(x) for x in Cl])

    sched = []
    for c in range(N_CORES):
        entry = {}
        for g in range(STREAMS):
            Cl = Cls[g]
            sumC = sum(Cl)
            R = PART * sumC
            trees = groups[g][c]
            pad = np.int16(-1 if DYN else 0)
            eidx = np.full(R, pad, np.int16)   # row -> embedding row (t*N + i)
            pidx = np.full(R, pad, np.int16)   # row -> parent state row
            cnt = np.zeros(L, np.int32)        # real rows per level (min 1)
            posmat = np.zeros((TREES_PER_STREAM, N), np.int32)  # node -> state row
            off = 0
            for l in range(L):
                base = PART * off
                j = 0
                for t, b in enumerate(trees):
                    for i in order[b][l]:
                        row = base + j
                        eidx[row] = t * N + i
                        posmat[t, i] = row
                        if l > 0:
                            pidx[row] = posmat[t, conn[b, i]]
                        j += 1
                assert j <= PART * Cl[l]
                if j == 0 and Cl[l] > 0:
                    eidx[base] = 0
                    pidx[base] = 0
                    j = 1
                cnt[l] = j
                off += Cl[l]

            def wrap(vals):
                # gather index layout: within a call of num_idxs n, index j
                # lives at [j%16, j//16]; replicate across the 8 groups of
                # 16 partitions. Calls slice per-level column blocks.
                out = np.zeros((PART, 8 * sumC), np.int16)
                o = 0
                for l in range(L):
                    n = PART * Cl[l]
                    block = vals[PART * o : PART * o + n].reshape(8 * Cl[l], 16).T  # [16, 8C]
                    for rep in range(8):
                        out[16 * rep : 16 * (rep + 1), 8 * o : 8 * (o + Cl[l])] = block
                    o += Cl[l]
                return out

            widx = np.full(R, pad, np.int16)   # row -> its own state row (for scatter wb)
            o2 = 0
            for l in range(L):
                nvalid = cnt[l]
                base = PART * o2
                widx[base : base + nvalid] = np.arange(base, base + nvalid, dtype=np.int16)
                o2 += Cl[l]
            entry[f"eidx{g}"] = wrap(eidx)
            entry[f"pidx{g}"] = wrap(pidx)
            entry[f"widx{g}"] = wrap(widx)
            entry[f"cnt{g}"] = cnt.reshape(1, L)
            entry[f"posmat{g}"] = posmat
            entry[f"trees{g}"] = list(trees)
        sched.append(entry)
    return L, Cls, sched


def _build_program(L, Cls):
    import concourse.bacc as bacc
    import concourse.mybir as mybir
    import concourse.tile as tile

    f32 = mybir.dt.float32
    i16 = mybir.dt.int16
    i32 = mybir.dt.int32
    Alu = mybir.AluOpType
    Act = mybir.ActivationFunctionType

    nc = bacc.Bacc("TRN2", debug=False)

    emb_t, eidx_t, pidx_t, cnt_t, state_t, widx_t = [], [], [], [], [], []
    for g in range(STREAMS):
        sumC = sum(Cls[g])
        R = PART * sumC
        emb_t.append(nc.dram_tensor(f"emb{g}", [TREES_PER_STREAM * 2048, DIM], f32,
                                    kind="ExternalInput"))
        eidx_t.append(nc.dram_tensor(f"eidx{g}", [PART, 8 * sumC], i16,
                                     kind="ExternalInput"))
        pidx_t.append(nc.dram_tensor(f"pidx{g}", [PART, 8 * sumC], i16,
                                     kind="ExternalInput"))
        cnt_t.append(nc.dram_tensor(f"cnt{g}", [1, L], i32, kind="ExternalInput"))
        if WBSCATTER:
            widx_t.append(nc.dram_tensor(f"widx{g}", [PART, 8 * sumC], i16,
                                         kind="ExternalInput"))
        state_t.append(nc.dram_tensor(f"state{g}", [R, DIM], f32,
                                      kind="ExternalOutput"))
    fake_t = None
    if ABLATE == "nodep":
        fake_t = nc.dram_tensor("fake", [PART * max(sum(C) for C in Cls), DIM], f32)

    with tile.TileContext(nc) as tc:
        from contextlib import ExitStack
        stack = ExitStack()
        pools = []
        for g in range(STREAMS):
            p = {
                "X": stack.enter_context(tc.tile_pool(name=f"X{g}", bufs=XBUFS)),
                "P": stack.enter_context(tc.tile_pool(name=f"P{g}", bufs=PBUFS)),
                "D": stack.enter_context(tc.tile_pool(name=f"D{g}", bufs=DBUFS)),
                "H": stack.enter_context(tc.tile_pool(name=f"H{g}", bufs=HBUFS)),
                "S": stack.enter_context(tc.tile_pool(name=f"S{g}", bufs=2)),
                "I": stack.enter_context(tc.tile_pool(name=f"I{g}", bufs=1)),
            }
            pools.append(p)

        # preload index arrays, allocate junk tiles
        idxs = []
        for g in range(STREAMS):
            sumC = sum(Cls[g])
            ei = pools[g]["I"].tile([PART, 8 * sumC], i16, tag=f"ei{g}")
            pi = pools[g]["I"].tile([PART, 8 * sumC], i16, tag=f"pi{g}")
            jt = pools[g]["I"].tile([PART, DIM], f32, tag=f"jt{g}")   # dot-product junk out
            nc.sync.dma_start(ei[:, :], eidx_t[g][:, :])
            nc.sync.dma_start(pi[:, :], pidx_t[g][:, :])
            wi = None
            if WBSCATTER:
                wi = pools[g]["I"].tile([PART, 8 * sumC], i16, tag=f"wi{g}")
                nc.sync.dma_start(wi[:, :], widx_t[g][:, :])
            cr = None
            if DYN:
                ct = pools[g]["I"].tile([1, L], i32, tag=f"ct{g}")
                nc.sync.dma_start(ct[:, :], cnt_t[g][:, :])
                # one register per level: reusing one would be a WAR hazard
                # under Tile reordering (gather reads reg at exec time)
                regs = [nc.gpsimd.alloc_register(f"cnt{g}_{l}") for l in range(L)]
                cr = (ct, regs)
            idxs.append((ei, pi, jt, cr, wi))

        Luse = min(L, MAXLEV) if MAXLEV else L
        STAGGER = os.environ.get("K_STAGGER", "0") == "1"
        for _rep in range(REPEAT):
          offs = [0 for _ in range(STREAMS)]
          if STAGGER:
            # emit (g, level) waves with stream g delayed by g levels, so the
            # streams' DMA/compute phases interleave rather than collide
            waves = []
            for w in range(Luse + STREAMS - 1):
                for g in range(STREAMS):
                    l = w - g
                    if 0 <= l < Luse:
                        waves.append((l, g))
            order = waves
          else:
            order = [(l, g) for l in range(Luse) for g in range(STREAMS)]
          for l, g in order:
            if True:
                C = Cls[g][l]
                if C == 0:
                    continue
                off = offs[g]
                offs[g] += C
                ei, pi, jt, cr, wi = idxs[g]
                p = pools[g]
                n = PART * C
                if DYN:
                    ct, regs = cr
                    nc.gpsimd.reg_load(regs[l], ct[0:1, l : l + 1])
                    nreg = regs[l]
                else:
                    nreg = n

                X = p["X"].tile([PART, C, DIM], f32, tag=f"X{g}")
                H = p["H"].tile([PART, C, DIM], f32, tag=f"H{g}")

                nc.gpsimd.dma_gather(
                    X[:, :, :], emb_t[g][:, :],
                    ei[:, 8 * off : 8 * (off + C)], n, nreg, DIM,
                    single_packet=SINGLE_PACKET)

                if l == 0:
                    nc.scalar.activation(H[:, :, :], X[:, :, :], Act.Copy)
                elif ABLATE == "nocompute":
                    P = p["P"].tile([PART, C, DIM], f32, tag=f"P{g}")
                    nc.gpsimd.dma_gather(
                        P[:, :, :], state_t[g][:, :],
                        pi[:, 8 * off : 8 * (off + C)], n, n, DIM)
                    nc.scalar.activation(H[:, :, :], P[:, :, :], Act.Copy)
                else:
                    P = p["P"].tile([PART, C, DIM], f32, tag=f"P{g}")
                    D = p["D"].tile([PART, C, DIM], f32, tag=f"D{g}")
                    dp = p["S"].tile([PART, C], f32, tag=f"dp{g}")
                    wh = p["S"].tile([PART, C], f32, tag=f"wh{g}")

                    gsrc = emb_t[g] if "pgemb" in SKIP else (
                        fake_t if ABLATE == "nodep" else state_t[g])
                    nc.gpsimd.dma_gather(
                        P[:, :, :], gsrc[:, :],
                        pi[:, 8 * off : 8 * (off + C)], n, nreg, DIM,
                        single_packet=SINGLE_PACKET)

                    # D = h_p - x
                    if "tt" in SKIP:
                        nc.scalar.activation(D[:, :, :], P[:, :, :], Act.Copy)
                    else:
                        nc.vector.tensor_tensor(D[:, :, :], P[:, :, :], X[:, :, :],
                                                Alu.subtract)
                    # z = <x, D> = <h_p, x> - <x, x>   (per chunk, fused mul+sum)
                    if "dotstt" in SKIP:
                        nc.vector.memset(dp[:, :], 0.0)
                    else:
                        for k in range(C):
                            nc.vector.scalar_tensor_tensor(
                                jt[:, :], X[:, k, :], 0.0, D[:, k, :],
                                Alu.bypass, Alu.mult,
                                accum_out=dp[:, k : k + 1])
                    # w = sigmoid(z) = alpha/(alpha+beta)
                    nc.scalar.activation(wh[:, :], dp[:, :], Act.Sigmoid)
                    # h = w*D + x
                    if "stt" in SKIP:
                        nc.scalar.activation(H[:, :, :], X[:, :, :], Act.Copy)
                    else:
                        for k in range(C):
                            nc.vector.scalar_tensor_tensor(
                                H[:, k, :], D[:, k, :], wh[:, k : k + 1], X[:, k, :],
                                Alu.mult, Alu.add)

                if WBSCATTER:
                    nc.gpsimd.dma_scatter_add(
                        state_t[g][:, :], H[:, :, :],
                        wi[:, 8 * off : 8 * (off + C)], n, nreg, DIM,
                        single_packet=SINGLE_PACKET)
                else:
                    dst = state_t[g][PART * off : PART * (off + C)].rearrange(
                        "(c p) e -> p c e", p=PART)
                    nc.sync.dma_start(dst, H[:, :, :])

        stack.close()

    nc.compile()
    return nc


def kernel(tree_embedding, node_connection, node_mask=None):
    import sys
    if "/opt/trn_rl_repo" not in sys.path:
        sys.path.insert(0, "/opt/trn_rl_repo")
    from concourse.bass_utils import run_bass_kernel_spmd

    emb = np.ascontiguousarray(np.asarray(tree_embedding, dtype=np.float32))
    conn = np.asarray(node_connection).astype(np.int32)
    B, N, D = emb.shape
    assert D == DIM and B == N_CORES * STREAMS * TREES_PER_STREAM

    L, Cls, sched = _build_schedule(conn)
    nc = _build_program(L, Cls)

    in_maps = []
    for c in range(N_CORES):
        m = {}
        for g in range(STREAMS):
            trees = sched[c][f"trees{g}"]
            m[f"emb{g}"] = emb[trees].reshape(TREES_PER_STREAM * N, DIM)
            m[f"eidx{g}"] = sched[c][f"eidx{g}"]
            m[f"pidx{g}"] = sched[c][f"pidx{g}"]
            if DYN:
                m[f"cnt{g}"] = sched[c][f"cnt{g}"]
            if WBSCATTER:
                m[f"widx{g}"] = sched[c][f"widx{g}"]
        in_maps.append(m)

    res = run_bass_kernel_spmd(nc, in_maps, list(range(N_CORES)))

    out = np.empty((B, N, DIM), np.float32)
    for c in range(N_CORES):
        for g in range(STREAMS):
            state = res.results[c][f"state{g}"]
            posmat = sched[c][f"posmat{g}"]
            for t, b in enumerate(sched[c][f"trees{g}"]):
                out[b] = state[posmat[t]]
    return out



# revision 42
# speedup vs baseline: 2.3279x; 2.3279x over previous
"""Trainium2 Bass kernel for nn_BareDotProdAttnEncoder (tree scan, gnn_message_passing).

Reference semantics (per batch element b):
  h_0 = x_0
  for i in 1..N-1:
      p = parent[i]  (p < i)
      alpha = exp(<h_p, x_i>); beta = exp(<x_i, x_i>)
      h_i = (alpha*h_p + beta*x_i) / (alpha + beta + 1e-15)

Equivalent form used on device:
  w = sigmoid(<h_p, x_i> - <x_i, x_i>) = sigmoid(<x_i, h_p - x_i>)
  h_i = w*(h_p - x_i) + x_i

Strategy: parent[i] < i uniform means depth(parent)+1 = depth(node), so all
nodes of one level are independent given the previous level. Host computes the
level schedule, PRE-SORTS the embeddings into level-packed order (so X loads
are contiguous DMA, no gather), and converts everything to bf16 (rel-err gate
is 2e-2; bf16 keeps us ~1e-3). Device loop per level: contiguous X load,
SWDGE dma_gather of parent h from the previous level's block of the persistent
HBM state buffer, DVE subtract/dot/blend + Act sigmoid, contiguous writeback.
Host inverse-permutes the returned state into reference node order (fp32).

Sharding: pure data parallelism over the batch; each of the 8 cores owns
4 trees, processed as STREAMS independent streams.
"""

import os
import numpy as np
import ml_dtypes

BF16 = ml_dtypes.bfloat16

N_CORES = 8
STREAMS = int(os.environ.get("K_STREAMS", "1"))
TREES_PER_STREAM = 4 // STREAMS
DIM = 512
PART = 128
XBUFS = int(os.environ.get("K_XBUFS", "3"))
PBUFS = int(os.environ.get("K_PBUFS", "2"))
DBUFS = int(os.environ.get("K_DBUFS", "2"))
HBUFS = int(os.environ.get("K_HBUFS", "2"))
MAXLEV = int(os.environ.get("K_MAXLEV", "0"))  # 0 = all levels
REPEAT = int(os.environ.get("K_REPEAT", "1"))
DYN = os.environ.get("K_DYN", "1") == "1"  # dynamic gather counts (skip pad traffic)
SINGLE_PACKET = os.environ.get("K_SINGLEPKT", "1") == "1"
STAGGER = os.environ.get("K_STAGGER", "0") == "1"
SUBENG = os.environ.get("K_SUBENG", "vector")  # engine for D = P - X
DOTENG = os.environ.get("K_DOTENG", "vector")  # vector|gpsimd|alt (per-chunk alt)
BLENDENG = os.environ.get("K_BLENDENG", "vector")
# alpha: D=P-X sub + dot<X,D> + TSP blend (all DVE-ish)
# beta:  host nn=<x,x>; dot<X,P> on DVE; blend = Diag(w)@P + Diag(1-w)@X on PE
ARCH = os.environ.get("K_ARCH", "beta")
DGENG = os.environ.get("K_DGENG", "gpsimd")  # engine building Diag(w) tiles
HCOPYENG = os.environ.get("K_HCOPYENG", "scalar")  # PSUM->SBUF H copy engine
PSUMBUFS = int(os.environ.get("K_PSUMBUFS", "3" if STREAMS < 3 else "2"))
# levels with C_l * C_{l-1} <= PERMMAX use PE permutation matmuls instead of
# the HBM gather round trip (kills small-level chain latency); 0 = off
PERMMAX = int(os.environ.get("K_PERMMAX", "4"))


def _compute_depths(conn):
    B, N = conn.shape
    depths = np.zeros((B, N), np.int32)
    bidx = np.arange(B)
    for i in range(1, N):
        depths[:, i] = depths[bidx, conn[:, i]] + 1
    return depths


def _assign_trees(S, B):
    """Group trees into (stream, core) slots to minimize total padded chunks.
    S: per-tree level-size matrix [B, L]. Returns groups[g][c] = tuple of trees.
    Deterministic local search (seeded)."""
    L = S.shape[1]
    tps = TREES_PER_STREAM
    nslots = B // tps  # STREAMS * N_CORES
    nat = [tuple(range(tps * s, tps * (s + 1))) for s in range(nslots)]

    def cost(assign):
        tot = 0
        for g in range(STREAMS):
            lv = np.zeros(L, np.int64)
            for c in range(N_CORES):
                grp = assign[g * N_CORES + c]
                n = np.sum(S[list(grp)], axis=0)
                lv = np.maximum(lv, (n + PART - 1) // PART)
            tot += lv.sum()
        return int(tot)

    if os.environ.get("K_NATASSIGN", "0") == "1":
        return [[nat[g * N_CORES + c] for c in range(N_CORES)] for g in range(STREAMS)]
    rng = np.random.default_rng(12345)
    cur = [list(p) for p in nat]
    cc = cost([tuple(p) for p in cur])
    best, bc = [tuple(p) for p in cur], cc
    for _ in range(20000):
        a = int(rng.integers(0, nslots)); b2 = int(rng.integers(0, nslots))
        if a == b2:
            continue
        i = int(rng.integers(0, tps)); j = int(rng.integers(0, tps))
        cur[a][i], cur[b2][j] = cur[b2][j], cur[a][i]
        c2 = cost([tuple(p) for p in cur])
        if c2 <= cc:
            cc = c2
            if c2 < bc:
                best, bc = [tuple(p) for p in cur], c2
        else:
            cur[a][i], cur[b2][j] = cur[b2][j], cur[a][i]
    return [[best[g * N_CORES + c] for c in range(N_CORES)] for g in range(STREAMS)]


def _build_schedule(conn):
    """Host-side schedule: level structure, per-core index arrays, maps.

    Returns (L, Cls, sched) where
      L: number of levels
      Cls[g]: list of per-level chunk counts (uniform across cores)
      sched[c]: dict with per-core input arrays + posmat for assembly
    """
    B, N = conn.shape
    depths = _compute_depths(conn)
    L = int(depths.max()) + 1

    # node lists per (batch, level), ordered by node id (stable)
    order = [[np.nonzero(depths[b] == l)[0] for l in range(L)] for b in range(B)]

    S = np.zeros((B, L), np.int64)
    for b in range(B):
        S[b] = np.bincount(depths[b], minlength=L)
    groups = _assign_trees(S, B)  # groups[g][c] = tree tuple

    # uniform chunk capacities per stream
    Cls = []
    for g in range(STREAMS):
        Cl = np.zeros(L, np.int64)
        for c in range(N_CORES):
            trees = groups[g][c]
            for l in range(L):
                n = sum(len(order[b][l]) for b in trees)
                Cl[l] = max(Cl[l], (n + PART - 1) // PART)
        Cls.append([int(x) for x in Cl])

    sched = []
    for c in range(N_CORES):
        entry = {}
        for g in range(STREAMS):
            Cl = Cls[g]
            sumC = sum(Cl)
            R = PART * sumC
            trees = groups[g][c]
            # levels eligible for the PE permutation path (no HBM gather)
            plv = set(l for l in range(1, L)
                      if Cl[l] * Cl[l - 1] <= PERMMAX and Cl[l] > 0)
            pad = np.int16(-1 if DYN else 0)
            eidx = np.full(R, -1, np.int32)     # row -> embedding row (t*N + i)
            pidx = np.full(R, pad, np.int16)    # row -> parent row REL. to prev level
            cnt = np.zeros(L, np.int32)         # real rows per level (min 1)
            posmat = np.zeros((TREES_PER_STREAM, N), np.int32)  # node -> state row
            off = 0
            prev_base = 0
            for l in range(L):
                base = PART * off
                j = 0
                for t, b in enumerate(trees):
                    for i in order[b][l]:
                        row = base + j
                        eidx[row] = t * N + i
                        posmat[t, i] = row
                        if l > 0:
                            pidx[row] = posmat[t, conn[b, i]] - prev_base
                        j += 1
                assert j <= PART * Cl[l]
                if j == 0 and Cl[l] > 0:
                    pidx[base] = 0
                    j = 1
                cnt[l] = j
                # a gather level feeding a perm level must produce FINITE pad
                # rows (the perm matmul computes 0*pad and NaN would poison
                # real rows): gather pads from prev row 0 instead of skipping
                if (l + 1) in plv and l not in plv and l > 0:
                    pidx[base + j : base + PART * Cl[l]] = 0
                prev_base = base
                off += Cl[l]

            def wrap(vals):
                # gather index layout: within a call of num_idxs n, index j
                # lives at [j%16, j//16]; replicate across the 8 groups of
                # 16 partitions. Calls slice per-level column blocks.
                out = np.zeros((PART, 8 * sumC), np.int16)
                o = 0
                for l in range(L):
                    n = PART * Cl[l]
                    block = vals[PART * o : PART * o + n].reshape(8 * Cl[l], 16).T  # [16, 8C]
                    for rep in range(8):
                        out[16 * rep : 16 * (rep + 1), 8 * o : 8 * (o + Cl[l])] = block
                    o += Cl[l]
                return out

            # permutation tiles for small levels (PE path): for level l with
            # C_l*C_{l-1} <= PERMMAX, all (m, c) out/in chunk pairs, each a
            # [128, 128] bf16 matrix T[k, j] = 1 iff parent(out row m*128+j)
            # == in row c*128+k (rows relative to level bases)
            permlevs = sorted(plv)
            ptiles = []
            off = 0
            lvl_base = np.cumsum([0] + list(Cl[:-1])) * PART
            for l in permlevs:
                base = lvl_base[l]
                for m in range(Cl[l]):
                    for cc in range(Cl[l - 1]):
                        T = np.zeros((PART, PART), BF16)
                        rel = pidx[base + m * PART : base + (m + 1) * PART]
                        for j in range(PART):
                            pr = int(rel[j])
                            if cc * PART <= pr < (cc + 1) * PART:
                                T[pr - cc * PART, j] = 1
                        ptiles.append(T)
            entry[f"perm{g}"] = (np.concatenate(ptiles, axis=1)
                                 if ptiles else np.zeros((PART, 0), BF16))
            entry[f"permlevs{g}"] = permlevs
            entry[f"pidx{g}"] = wrap(pidx)
            entry[f"cnt{g}"] = cnt.reshape(1, L)
            entry[f"posmat{g}"] = posmat
            entry[f"trees{g}"] = list(trees)
            entry[f"eidxlin{g}"] = eidx  # linear, for host presort
        sched.append(entry)
    return L, Cls, sched


def _presort_nn(embS):
    """nn[p, c] = <x,x> of state row c*128+p, from the presorted bf16 emb."""
    R = embS.shape[0]
    nn = (embS.astype(np.float32) ** 2).sum(axis=1)  # [R]
    return np.ascontiguousarray(nn.reshape(R // PART, PART).T)  # [128, sumC]


def _presort_emb(emb_bf, sched, c, g, Cls):
    """Level-packed bf16 embedding matrix for (core, stream): [R, DIM]."""
    sumC = sum(Cls[g])
    R = PART * sumC
    trees = sched[c][f"trees{g}"]
    src = emb_bf[trees].reshape(-1, DIM)   # [TPS*N, DIM]
    eidx = sched[c][f"eidxlin{g}"]
    out = np.zeros((R, DIM), BF16)
    m = eidx >= 0
    out[m] = src[eidx[m]]
    return out


def _build_program(L, Cls, permlevs=None):
    import concourse.bacc as bacc
    import concourse.mybir as mybir
    import concourse.tile as tile

    permlevs = permlevs or [[] for _ in range(STREAMS)]

    bf16 = mybir.dt.bfloat16
    f32 = mybir.dt.float32
    i16 = mybir.dt.int16
    i32 = mybir.dt.int32
    Alu = mybir.AluOpType
    Act = mybir.ActivationFunctionType

    nc = bacc.Bacc("TRN2", debug=False)

    emb_t, pidx_t, cnt_t, state_t, nn_t = [], [], [], [], []
    for g in range(STREAMS):
        sumC = sum(Cls[g])
        R = PART * sumC
        emb_t.append(nc.dram_tensor(f"embS{g}", [R, DIM], bf16, kind="ExternalInput"))
        pidx_t.append(nc.dram_tensor(f"pidx{g}", [PART, 8 * sumC], i16,
                                     kind="ExternalInput"))
        cnt_t.append(nc.dram_tensor(f"cnt{g}", [1, L], i32, kind="ExternalInput"))
        state_t.append(nc.dram_tensor(f"state{g}", [R, DIM], bf16,
                                      kind="ExternalOutput"))
    perm_t, npairs = [], []
    for g in range(STREAMS):
        np_g = sum(Cls[g][l] * Cls[g][l - 1] for l in permlevs[g])
        npairs.append(np_g)
        perm_t.append(nc.dram_tensor(f"perm{g}", [PART, PART * np_g], bf16,
                                     kind="ExternalInput") if np_g else None)

    with tile.TileContext(nc) as tc:
        from contextlib import ExitStack
        stack = ExitStack()
        pools = []
        for g in range(STREAMS):
            p = {
                "X": stack.enter_context(tc.tile_pool(name=f"X{g}", bufs=XBUFS)),
                "P": stack.enter_context(tc.tile_pool(name=f"P{g}", bufs=PBUFS)),
                "D": stack.enter_context(tc.tile_pool(name=f"D{g}", bufs=DBUFS)),
                "H": stack.enter_context(tc.tile_pool(name=f"H{g}", bufs=HBUFS)),
                "S": stack.enter_context(tc.tile_pool(name=f"S{g}", bufs=2)),
                "I": stack.enter_context(tc.tile_pool(name=f"I{g}", bufs=1)),
            }
            if permlevs[g]:
                p["PS"] = stack.enter_context(
                    tc.tile_pool(name=f"PS{g}", bufs=PSUMBUFS, space="PSUM"))
            pools.append(p)

        # preload index arrays, allocate junk tiles
        idxs = []
        for g in range(STREAMS):
            sumC = sum(Cls[g])
            pi = pools[g]["I"].tile([PART, 8 * sumC], i16, tag=f"pi{g}")
            # per-engine junk outputs for the accumulating dot (avoid
            # cross-engine WAW serialization on a shared junk tile)
            jtv = pools[g]["I"].tile([PART, DIM], bf16, tag=f"jtv{g}")
            jtp = pools[g]["I"].tile([PART, DIM], bf16, tag=f"jtp{g}")
            jt = {nc.vector: jtv, nc.gpsimd: jtp}
            nc.sync.dma_start(pi[:, :], pidx_t[g][:, :])
            pm = None
            if npairs[g]:
                pm = pools[g]["I"].tile([PART, PART * npairs[g]], bf16,
                                        tag=f"pm{g}")
                nc.sync.dma_start(pm[:, :], perm_t[g][:, :])
            cr = None
            if DYN:
                ct = pools[g]["I"].tile([1, L], i32, tag=f"ct{g}")
                nc.sync.dma_start(ct[:, :], cnt_t[g][:, :])
                # one register per level: reusing one would be a WAR hazard
                # under Tile reordering (gather reads reg at exec time)
                regs = [nc.gpsimd.alloc_register(f"cnt{g}_{l}") for l in range(L)]
                cr = (ct, regs)
            idxs.append((pi, jt, cr, pm))

        Luse = min(L, MAXLEV) if MAXLEV else L
        Hprev = [None for _ in range(STREAMS)]
        pair_off = [0 for _ in range(STREAMS)]
        for _rep in range(REPEAT):
          offs = [0 for _ in range(STREAMS)]
          prev_offs = [0 for _ in range(STREAMS)]
          pair_off = [0 for _ in range(STREAMS)]
          if STAGGER and STREAMS > 1:
            waves = []
            for w in range(Luse + STREAMS - 1):
                for g in range(STREAMS):
                    l = w - g
                    if 0 <= l < Luse:
                        waves.append((l, g))
            order = waves
          else:
            order = [(l, g) for l in range(Luse) for g in range(STREAMS)]
          for l, g in order:
            C = Cls[g][l]
            if C == 0:
                continue
            off = offs[g]
            offs[g] += C
            pi, jt, cr, pm = idxs[g]
            p = pools[g]
            n = PART * C
            is_perm = l in permlevs[g]

            X = p["X"].tile([PART, C, DIM], bf16, tag=f"X{g}")
            xsrc = emb_t[g][PART * off : PART * (off + C)].rearrange(
                "(c p) e -> p c e", p=PART)
            nc.sync.dma_start(X[:, :, :], xsrc)

            if l == 0:
                # h = x for roots: X tile doubles as H_0
                dst = state_t[g][0 : PART * C].rearrange(
                    "(c p) e -> p c e", p=PART)
                nc.sync.dma_start(dst, X[:, :, :])
                Hprev[g] = X
                prev_offs[g] = off
                continue

            Cp = Cls[g][l - 1]
            poff = prev_offs[g]

            H = p["H"].tile([PART, C, DIM], bf16, tag=f"H{g}")
            dp = p["S"].tile([PART, C], f32, tag=f"dp{g}")
            wh = p["S"].tile([PART, C], f32, tag=f"wh{g}")

            P = None
            psl = None
            if is_perm:
                # P = Perm @ H_{l-1} on the PE from the previous level's
                # SBUF tile; no HBM round trip on the critical path
                psl = []
                po = pair_off[g]
                for m in range(C):
                    pst = p["PS"].tile([PART, DIM], f32, tag=f"psp{g}")
                    psl.append(pst)
                    for cc in range(Cp):
                        t0 = PART * (po + m * Cp + cc)
                        nc.tensor.matmul(
                            pst[:, :], pm[:, t0 : t0 + PART],
                            Hprev[g][:, cc, :],
                            start=(cc == 0), stop=(cc == Cp - 1))
                pair_off[g] = po + C * Cp
            else:
                # gather levels feeding a perm level run full-count (their
                # pidx pads were set to 0 by the schedule)
                full = (l + 1) in permlevs[g]
                if DYN and not full:
                    ct, regs = cr
                    nc.gpsimd.reg_load(regs[l], ct[0:1, l : l + 1])
                    nreg = regs[l]
                else:
                    nreg = n
                P = p["P"].tile([PART, C, DIM], bf16, tag=f"P{g}")
                # gather parent h from the PREVIOUS level's block only
                # (indices are relative to that block)
                gsrc = state_t[g][PART * poff : PART * (poff + Cp), :]
                nc.gpsimd.dma_gather(
                    P[:, :, :], gsrc,
                    pi[:, 8 * off : 8 * (off + C)], n, nreg, DIM,
                    single_packet=SINGLE_PACKET)

            def pick(which, k):
                mode = {"sub": SUBENG, "dot": DOTENG, "blend": BLENDENG}[which]
                if mode == "alt":
                    return nc.vector if k % 2 == 0 else nc.gpsimd
                if mode.startswith("pool1of"):  # every Nth chunk on Pool
                    return nc.gpsimd if k % int(mode[7:]) == 0 else nc.vector
                return nc.vector if mode == "vector" else nc.gpsimd

            par = (lambda k: psl[k][:, :]) if is_perm else (lambda k: P[:, k, :])
            D = p["D"].tile([PART, C, DIM], bf16, tag=f"D{g}")
            # D = h_p - x
            if is_perm:
                for k in range(C):
                    pick("sub", k).tensor_tensor(
                        D[:, k, :], par(k), X[:, k, :], Alu.subtract)
            else:
                pick("sub", 0).tensor_tensor(D[:, :, :], P[:, :, :],
                                             X[:, :, :], Alu.subtract)
            # z = <x, D> = <h_p, x> - <x, x>   (per chunk, fused mul+sum)
            for k in range(C):
                deng = pick("dot", k)
                deng.scalar_tensor_tensor(
                    jt[deng][:, :], X[:, k, :], 0.0, D[:, k, :],
                    Alu.bypass, Alu.mult,
                    accum_out=dp[:, k : k + 1])
            # w = sigmoid(z) = alpha/(alpha+beta)
            nc.scalar.activation(wh[:, :], dp[:, :], Act.Sigmoid)
            # h = w*D + x
            for k in range(C):
                pick("blend", k).scalar_tensor_tensor(
                    H[:, k, :], D[:, k, :], wh[:, k : k + 1], X[:, k, :],
                    Alu.mult, Alu.add)

            dst = state_t[g][PART * off : PART * (off + C)].rearrange(
                "(c p) e -> p c e", p=PART)
            nc.sync.dma_start(dst, H[:, :, :])
            Hprev[g] = H
            prev_offs[g] = off

        stack.close()

    nc.compile()
    return nc


def kernel(tree_embedding, node_connection, node_mask=None):
    import sys
    if "/opt/trn_rl_repo" not in sys.path:
        sys.path.insert(0, "/opt/trn_rl_repo")
    from concourse.bass_utils import run_bass_kernel_spmd

    emb = np.asarray(tree_embedding, dtype=np.float32)
    emb_bf = emb.astype(BF16)
    conn = np.asarray(node_connection).astype(np.int32)
    B, N, D = emb.shape
    assert D == DIM and B == N_CORES * STREAMS * TREES_PER_STREAM

    L, Cls, sched = _build_schedule(conn)
    permlevs = [sched[0][f"permlevs{g}"] for g in range(STREAMS)]
    nc = _build_program(L, Cls, permlevs)

    in_maps = []
    for c in range(N_CORES):
        m = {}
        for g in range(STREAMS):
            embS = _presort_emb(emb_bf, sched, c, g, Cls)
            m[f"embS{g}"] = embS
            m[f"pidx{g}"] = sched[c][f"pidx{g}"]
            if DYN:
                m[f"cnt{g}"] = sched[c][f"cnt{g}"]
            if sched[c][f"perm{g}"].shape[1]:
                m[f"perm{g}"] = sched[c][f"perm{g}"]
        in_maps.append(m)

    res = run_bass_kernel_spmd(nc, in_maps, list(range(N_CORES)))

    out = np.empty((B, N, DIM), np.float32)
    for c in range(N_CORES):
        for g in range(STREAMS):
            state = np.asarray(res.results[c][f"state{g}"]).astype(np.float32)
            posmat = sched[c][f"posmat{g}"]
            for t, b in enumerate(sched[c][f"trees{g}"]):
                out[b] = state[posmat[t]]
    return out


# revision 62
# speedup vs baseline: 3.1273x; 1.3434x over previous
"""Trainium2 Bass kernel for nn_BareDotProdAttnEncoder (tree scan, gnn_message_passing).

Reference semantics (per batch element b):
  h_0 = x_0
  for i in 1..N-1:
      p = parent[i]  (p < i)
      alpha = exp(<h_p, x_i>); beta = exp(<x_i, x_i>)
      h_i = (alpha*h_p + beta*x_i) / (alpha + beta + 1e-15)

Equivalent form used on device:
  w = sigmoid(<h_p, x_i> - <x_i, x_i>) = sigmoid(<x_i, h_p - x_i>)
  h_i = w*(h_p - x_i) + x_i

Strategy: parent[i] < i uniform means depth(parent)+1 = depth(node), so all
nodes of one level are independent given the previous level. Host computes the
level schedule, PRE-SORTS the embeddings into level-packed order (so X loads
are contiguous DMA, no gather), and converts everything to bf16 (rel-err gate
is 2e-2; bf16 keeps us ~1e-3). Device loop per level: contiguous X load,
SWDGE dma_gather of parent h from the previous level's block of the persistent
HBM state buffer, DVE subtract/dot/blend + Act sigmoid, contiguous writeback.
Host inverse-permutes the returned state into reference node order (fp32).

Sharding: pure data parallelism over the batch; each of the 8 cores owns
4 trees, processed as STREAMS independent streams.
"""

import os
import numpy as np
import ml_dtypes

BF16 = ml_dtypes.bfloat16

N_CORES = 8
STREAMS = int(os.environ.get("K_STREAMS", "1"))
TREES_PER_STREAM = 4 // STREAMS
DIM = 512
PART = 128
XBUFS = int(os.environ.get("K_XBUFS", "3"))
PBUFS = int(os.environ.get("K_PBUFS", "2"))
DBUFS = int(os.environ.get("K_DBUFS", "2"))
HBUFS = int(os.environ.get("K_HBUFS", "2"))
MAXLEV = int(os.environ.get("K_MAXLEV", "0"))  # 0 = all levels
REPEAT = int(os.environ.get("K_REPEAT", "1"))
DYN = os.environ.get("K_DYN", "1") == "1"  # dynamic gather counts (skip pad traffic)
SINGLE_PACKET = os.environ.get("K_SINGLEPKT", "1") == "1"
STAGGER = os.environ.get("K_STAGGER", "0") == "1"
SUBENG = os.environ.get("K_SUBENG", "vector")  # engine for D = P - X
DOTENG = os.environ.get("K_DOTENG", "vector")  # vector|gpsimd|alt (per-chunk alt)
BLENDENG = os.environ.get("K_BLENDENG", "vector")
# alpha: D=P-X sub + dot<X,D> + TSP blend (all DVE-ish)
# beta:  host nn=<x,x>; dot<X,P> on DVE; blend = Diag(w)@P + Diag(1-w)@X on PE
ARCH = os.environ.get("K_ARCH", "beta")
DGENG = os.environ.get("K_DGENG", "gpsimd")  # engine building Diag(w) tiles
HCOPYENG = os.environ.get("K_HCOPYENG", "scalar")  # PSUM->SBUF H copy engine
PSUMBUFS = int(os.environ.get("K_PSUMBUFS", "3" if STREAMS < 3 else "2"))
# levels with C_l * C_{l-1} <= PERMMAX use PE permutation matmuls instead of
# the HBM gather round trip (kills small-level chain latency); 0 = off
PERMMAX = int(os.environ.get("K_PERMMAX", "4"))
# split dot/blend into TT(2x) + tensor_scalar(4x) pairs instead of one
# TensorScalarPtr (no fast mode) -- ~20% fewer DVE cycles
DOT4X = os.environ.get("K_DOT4X", "0") == "1"
BLEND4X = os.environ.get("K_BLEND4X", "0") == "1"
# BLEND3: drop D entirely; dot = <X,P> - nn (host nn), blend =
# (P*w) + (X*(1-w)) via two 4x tensor_scalars + one 2x TT
BLEND3 = os.environ.get("K_BLEND3", "0") == "1"


def _compute_depths(conn):
    B, N = conn.shape
    depths = np.zeros((B, N), np.int32)
    bidx = np.arange(B)
    for i in range(1, N):
        depths[:, i] = depths[bidx, conn[:, i]] + 1
    return depths


def _assign_trees(S, B):
    """Group trees into (stream, core) slots to minimize total padded chunks.
    S: per-tree level-size matrix [B, L]. Returns groups[g][c] = tuple of trees.
    Deterministic local search (seeded)."""
    L = S.shape[1]
    tps = TREES_PER_STREAM
    nslots = B // tps  # STREAMS * N_CORES
    nat = [tuple(range(tps * s, tps * (s + 1))) for s in range(nslots)]

    def cost(assign):
        tot = 0
        for g in range(STREAMS):
            lv = np.zeros(L, np.int64)
            for c in range(N_CORES):
                grp = assign[g * N_CORES + c]
                n = np.sum(S[list(grp)], axis=0)
                lv = np.maximum(lv, (n + PART - 1) // PART)
            tot += lv.sum()
        return int(tot)

    if os.environ.get("K_NATASSIGN", "0") == "1":
        return [[nat[g * N_CORES + c] for c in range(N_CORES)] for g in range(STREAMS)]

    def vcost(cur):  # cur: [nslots, tps] int array
        tot = 0
        for g in range(STREAMS):
            gs = S[cur[g * N_CORES : (g + 1) * N_CORES]].sum(axis=1)
            tot += int(np.ceil(gs / PART).max(axis=0).sum())
        return tot

    def anneal(seed, iters):
        rng = np.random.default_rng(seed)
        cur = rng.permutation(B).reshape(nslots, tps)
        cc = vcost(cur)
        best, bc = cur.copy(), cc
        for it in range(iters):
            T = 1.5 * (0.01 / 1.5) ** (it / iters)
            a, b2 = rng.integers(0, nslots, 2)
            i, j = rng.integers(0, tps, 2)
            if a == b2 and i == j:
                continue
            cur[a, i], cur[b2, j] = cur[b2, j], cur[a, i]
            c2 = vcost(cur)
            if c2 <= cc or rng.random() < np.exp((cc - c2) / max(T, 1e-9)):
                cc = c2
                if c2 < bc:
                    bc, best = c2, cur.copy()
            else:
                cur[a, i], cur[b2, j] = cur[b2, j], cur[a, i]
        return bc, best

    bc, best = min((anneal(s, 30000) for s in range(6)), key=lambda x: x[0])
    return [[tuple(int(t) for t in best[g * N_CORES + c]) for c in range(N_CORES)]
            for g in range(STREAMS)]


def _build_schedule(conn):
    """Host-side schedule: level structure, per-core index arrays, maps.

    Returns (L, Cls, sched) where
      L: number of levels
      Cls[g]: list of per-level chunk counts (uniform across cores)
      sched[c]: dict with per-core input arrays + posmat for assembly
    """
    B, N = conn.shape
    depths = _compute_depths(conn)
    L = int(depths.max()) + 1

    # node lists per (batch, level), ordered by node id (stable)
    order = [[np.nonzero(depths[b] == l)[0] for l in range(L)] for b in range(B)]

    S = np.zeros((B, L), np.int64)
    for b in range(B):
        S[b] = np.bincount(depths[b], minlength=L)
    groups = _assign_trees(S, B)  # groups[g][c] = tree tuple

    # uniform chunk capacities per stream
    Cls = []
    for g in range(STREAMS):
        Cl = np.zeros(L, np.int64)
        for c in range(N_CORES):
            trees = groups[g][c]
            for l in range(L):
                n = sum(len(order[b][l]) for b in trees)
                Cl[l] = max(Cl[l], (n + PART - 1) // PART)
        Cls.append([int(x) for x in Cl])

    sched = []
    for c in range(N_CORES):
        entry = {}
        for g in range(STREAMS):
            Cl = Cls[g]
            sumC = sum(Cl)
            R = PART * sumC
            trees = groups[g][c]
            # levels eligible for the PE permutation path (no HBM gather)
            plv = set(l for l in range(1, L)
                      if Cl[l] * Cl[l - 1] <= PERMMAX and Cl[l] > 0)
            pad = np.int16(-1 if DYN else 0)
            eidx = np.full(R, -1, np.int32)     # row -> embedding row (t*N + i)
            pidx = np.full(R, pad, np.int16)    # row -> parent row REL. to prev level
            cnt = np.zeros(L, np.int32)         # real rows per level (min 1)
            posmat = np.zeros((TREES_PER_STREAM, N), np.int32)  # node -> state row
            off = 0
            prev_base = 0
            for l in range(L):
                base = PART * off
                j = 0
                for t, b in enumerate(trees):
                    for i in order[b][l]:
                        row = base + j
                        eidx[row] = t * N + i
                        posmat[t, i] = row
                        if l > 0:
                            pidx[row] = posmat[t, conn[b, i]] - prev_base
                        j += 1
                assert j <= PART * Cl[l]
                if j == 0 and Cl[l] > 0:
                    pidx[base] = 0
                    j = 1
                cnt[l] = j
                # a gather level feeding a perm level must produce FINITE pad
                # rows (the perm matmul computes 0*pad and NaN would poison
                # real rows): gather pads from prev row 0 instead of skipping
                if (l + 1) in plv and l not in plv and l > 0:
                    pidx[base + j : base + PART * Cl[l]] = 0
                prev_base = base
                off += Cl[l]

            def wrap(vals):
                # gather index layout: within a call of num_idxs n, index j
                # lives at [j%16, j//16]; replicate across the 8 groups of
                # 16 partitions. Calls slice per-level column blocks.
                out = np.zeros((PART, 8 * sumC), np.int16)
                o = 0
                for l in range(L):
                    n = PART * Cl[l]
                    block = vals[PART * o : PART * o + n].reshape(8 * Cl[l], 16).T  # [16, 8C]
                    for rep in range(8):
                        out[16 * rep : 16 * (rep + 1), 8 * o : 8 * (o + Cl[l])] = block
                    o += Cl[l]
                return out

            # permutation tiles for small levels (PE path): for level l with
            # C_l*C_{l-1} <= PERMMAX, all (m, c) out/in chunk pairs, each a
            # [128, 128] bf16 matrix T[k, j] = 1 iff parent(out row m*128+j)
            # == in row c*128+k (rows relative to level bases)
            permlevs = sorted(plv)
            ptiles = []
            off = 0
            lvl_base = np.cumsum([0] + list(Cl[:-1])) * PART
            for l in permlevs:
                base = lvl_base[l]
                for m in range(Cl[l]):
                    for cc in range(Cl[l - 1]):
                        T = np.zeros((PART, PART), BF16)
                        rel = pidx[base + m * PART : base + (m + 1) * PART]
                        for j in range(PART):
                            pr = int(rel[j])
                            if cc * PART <= pr < (cc + 1) * PART:
                                T[pr - cc * PART, j] = 1
                        ptiles.append(T)
            entry[f"perm{g}"] = (np.concatenate(ptiles, axis=1)
                                 if ptiles else np.zeros((PART, 0), BF16))
            entry[f"permlevs{g}"] = permlevs
            entry[f"pidx{g}"] = wrap(pidx)
            entry[f"cnt{g}"] = cnt.reshape(1, L)
            entry[f"posmat{g}"] = posmat
            entry[f"trees{g}"] = list(trees)
            entry[f"eidxlin{g}"] = eidx  # linear, for host presort
        sched.append(entry)
    return L, Cls, sched


def _presort_nn(embS):
    """nn[p, c] = <x,x> of state row c*128+p, from the presorted bf16 emb."""
    R = embS.shape[0]
    nn = (embS.astype(np.float32) ** 2).sum(axis=1)  # [R]
    return np.ascontiguousarray(nn.reshape(R // PART, PART).T)  # [128, sumC]


def _presort_emb(emb_bf, sched, c, g, Cls):
    """Level-packed bf16 embedding matrix for (core, stream): [R, DIM]."""
    sumC = sum(Cls[g])
    R = PART * sumC
    trees = sched[c][f"trees{g}"]
    src = emb_bf[trees].reshape(-1, DIM)   # [TPS*N, DIM]
    eidx = sched[c][f"eidxlin{g}"]
    out = np.zeros((R, DIM), BF16)
    m = eidx >= 0
    out[m] = src[eidx[m]]
    return out


def _build_program(L, Cls, permlevs=None):
    import concourse.bacc as bacc
    import concourse.mybir as mybir
    import concourse.tile as tile

    permlevs = permlevs or [[] for _ in range(STREAMS)]

    bf16 = mybir.dt.bfloat16
    f32 = mybir.dt.float32
    i16 = mybir.dt.int16
    i32 = mybir.dt.int32
    Alu = mybir.AluOpType
    Act = mybir.ActivationFunctionType

    nc = bacc.Bacc("TRN2", debug=False)

    emb_t, pidx_t, cnt_t, state_t, nn_t = [], [], [], [], []
    for g in range(STREAMS):
        sumC = sum(Cls[g])
        R = PART * sumC
        emb_t.append(nc.dram_tensor(f"embS{g}", [R, DIM], bf16, kind="ExternalInput"))
        pidx_t.append(nc.dram_tensor(f"pidx{g}", [PART, 8 * sumC], i16,
                                     kind="ExternalInput"))
        cnt_t.append(nc.dram_tensor(f"cnt{g}", [1, L], i32, kind="ExternalInput"))
        state_t.append(nc.dram_tensor(f"state{g}", [R, DIM], bf16,
                                      kind="ExternalOutput"))
    ident_t = nc.dram_tensor("ident", [PART, PART], bf16, kind="ExternalInput")
    nn_t = []
    if BLEND3:
        for g in range(STREAMS):
            nn_t.append(nc.dram_tensor(f"nn{g}", [PART, sum(Cls[g])], f32,
                                       kind="ExternalInput"))
    perm_t, npairs = [], []
    for g in range(STREAMS):
        np_g = sum(Cls[g][l] * Cls[g][l - 1] for l in permlevs[g])
        npairs.append(np_g)
        perm_t.append(nc.dram_tensor(f"perm{g}", [PART, PART * np_g], bf16,
                                     kind="ExternalInput") if np_g else None)

    with tile.TileContext(nc) as tc:
        from contextlib import ExitStack
        stack = ExitStack()
        pools = []
        for g in range(STREAMS):
            p = {
                "X": stack.enter_context(tc.tile_pool(name=f"X{g}", bufs=XBUFS)),
                "P": stack.enter_context(tc.tile_pool(name=f"P{g}", bufs=PBUFS)),
                "D": stack.enter_context(tc.tile_pool(name=f"D{g}", bufs=DBUFS)),
                "H": stack.enter_context(tc.tile_pool(name=f"H{g}", bufs=HBUFS)),
                "S": stack.enter_context(tc.tile_pool(name=f"S{g}", bufs=2)),
                "I": stack.enter_context(tc.tile_pool(name=f"I{g}", bufs=1)),
            }
            if permlevs[g] or SUBENG == "pe":
                p["PS"] = stack.enter_context(
                    tc.tile_pool(name=f"PS{g}", bufs=PSUMBUFS, space="PSUM"))
            if BLENDENG.startswith("act") or BLEND4X or DOT4X or BLEND3:
                p["T"] = stack.enter_context(tc.tile_pool(name=f"T{g}", bufs=4))
            pools.append(p)

        # preload index arrays, allocate junk tiles
        ip = stack.enter_context(tc.tile_pool(name="ip", bufs=1))
        ident_sb = ip.tile([PART, PART], bf16, tag="ident")
        nc.sync.dma_start(ident_sb[:, :], ident_t[:, :])
        negident_sb = None
        if SUBENG == "pe":
            negident_sb = ip.tile([PART, PART], bf16, tag="negident")
            nc.vector.tensor_scalar(negident_sb[:, :], ident_sb[:, :], -1.0,
                                    None, Alu.mult)
        idxs = []
        for g in range(STREAMS):
            sumC = sum(Cls[g])
            pi = pools[g]["I"].tile([PART, 8 * sumC], i16, tag=f"pi{g}")
            # per-engine junk outputs for the accumulating dot (avoid
            # cross-engine WAW serialization on a shared junk tile)
            jtv = pools[g]["I"].tile([PART, DIM], bf16, tag=f"jtv{g}")
            jtp = pools[g]["I"].tile([PART, DIM], bf16, tag=f"jtp{g}")
            jt = {nc.vector: jtv, nc.gpsimd: jtp}
            nc.sync.dma_start(pi[:, :], pidx_t[g][:, :])
            nn_sb = None
            if BLEND3:
                nn_sb = pools[g]["I"].tile([PART, sumC], f32, tag=f"nn{g}")
                nc.sync.dma_start(nn_sb[:, :], nn_t[g][:, :])
            pm = None
            if npairs[g]:
                pm = pools[g]["I"].tile([PART, PART * npairs[g]], bf16,
                                        tag=f"pm{g}")
                nc.sync.dma_start(pm[:, :], perm_t[g][:, :])
            cr = None
            if DYN:
                ct = pools[g]["I"].tile([1, L], i32, tag=f"ct{g}")
                nc.sync.dma_start(ct[:, :], cnt_t[g][:, :])
                # one register per level: reusing one would be a WAR hazard
                # under Tile reordering (gather reads reg at exec time)
                regs = [nc.gpsimd.alloc_register(f"cnt{g}_{l}") for l in range(L)]
                cr = (ct, regs)
            idxs.append((pi, jt, cr, pm, nn_sb))

        Luse = min(L, MAXLEV) if MAXLEV else L
        Hprev = [None for _ in range(STREAMS)]
        pair_off = [0 for _ in range(STREAMS)]
        for _rep in range(REPEAT):
          offs = [0 for _ in range(STREAMS)]
          prev_offs = [0 for _ in range(STREAMS)]
          pair_off = [0 for _ in range(STREAMS)]
          if STAGGER and STREAMS > 1:
            waves = []
            for w in range(Luse + STREAMS - 1):
                for g in range(STREAMS):
                    l = w - g
                    if 0 <= l < Luse:
                        waves.append((l, g))
            order = waves
          else:
            order = [(l, g) for l in range(Luse) for g in range(STREAMS)]
          for l, g in order:
            C = Cls[g][l]
            if C == 0:
                continue
            off = offs[g]
            offs[g] += C
            pi, jt, cr, pm, nn_sb = idxs[g]
            p = pools[g]
            n = PART * C
            is_perm = l in permlevs[g]

            X = p["X"].tile([PART, C, DIM], bf16, tag=f"X{g}")
            xsrc = emb_t[g][PART * off : PART * (off + C)].rearrange(
                "(c p) e -> p c e", p=PART)
            nc.sync.dma_start(X[:, :, :], xsrc)

            if l == 0:
                # h = x for roots: X tile doubles as H_0
                dst = state_t[g][0 : PART * C].rearrange(
                    "(c p) e -> p c e", p=PART)
                nc.sync.dma_start(dst, X[:, :, :])
                Hprev[g] = X
                prev_offs[g] = off
                continue

            Cp = Cls[g][l - 1]
            poff = prev_offs[g]

            H = p["H"].tile([PART, C, DIM], bf16, tag=f"H{g}")
            dp = p["S"].tile([PART, C], f32, tag=f"dp{g}")
            wh = p["S"].tile([PART, C], f32, tag=f"wh{g}")

            P = None
            psl = None
            if is_perm:
                # P = Perm @ H_{l-1} on the PE from the previous level's
                # SBUF tile; no HBM round trip on the critical path
                psl = []
                po = pair_off[g]
                for m in range(C):
                    pst = p["PS"].tile([PART, DIM], f32, tag=f"psq{g}")
                    psl.append(pst)
                    for cc in range(Cp):
                        t0 = PART * (po + m * Cp + cc)
                        nc.tensor.matmul(
                            pst[:, :], pm[:, t0 : t0 + PART],
                            Hprev[g][:, cc, :],
                            start=(cc == 0), stop=(cc == Cp - 1))
                pair_off[g] = po + C * Cp
            else:
                # gather levels feeding a perm level run full-count (their
                # pidx pads were set to 0 by the schedule)
                full = (l + 1) in permlevs[g]
                if DYN and not full:
                    ct, regs = cr
                    nc.gpsimd.reg_load(regs[l], ct[0:1, l : l + 1])
                    nreg = regs[l]
                else:
                    nreg = n
                P = p["P"].tile([PART, C, DIM], bf16, tag=f"P{g}")
                # gather parent h from the PREVIOUS level's block only
                # (indices are relative to that block)
                gsrc = state_t[g][PART * poff : PART * (poff + Cp), :]
                nc.gpsimd.dma_gather(
                    P[:, :, :], gsrc,
                    pi[:, 8 * off : 8 * (off + C)], n, nreg, DIM,
                    single_packet=SINGLE_PACKET)

            def pick(which, k):
                mode = {"sub": SUBENG, "dot": DOTENG, "blend": BLENDENG}[which]
                if mode == "alt":
                    return nc.vector if k % 2 == 0 else nc.gpsimd
                if mode.startswith("pool1of"):  # every Nth chunk on Pool
                    return nc.gpsimd if k % int(mode[7:]) == 0 else nc.vector
                return nc.vector if mode == "vector" else nc.gpsimd

            par = (lambda k: psl[k][:, :]) if is_perm else (lambda k: P[:, k, :])

            if BLEND3 and not is_perm:
                # dot on P directly; blend via two 4x tensor_scalars + TT
                for k in range(C):
                    deng = pick("dot", k)
                    deng.scalar_tensor_tensor(
                        jt[deng][:, :], X[:, k, :], 0.0, P[:, k, :],
                        Alu.bypass, Alu.mult,
                        accum_out=dp[:, k : k + 1])
                z2 = p["S"].tile([PART, C], f32, tag=f"z2{g}")
                w2 = p["S"].tile([PART, C], f32, tag=f"wt{g}")
                nc.vector.tensor_tensor(z2[:, :], dp[:, :],
                                        nn_sb[:, off : off + C], Alu.subtract)
                nc.scalar.activation(wh[:, :], z2[:, :], Act.Sigmoid)
                nc.vector.tensor_scalar(w2[:, :], wh[:, :], -1.0, 1.0,
                                        Alu.mult, Alu.add)
                for k in range(C):
                    T1 = p["T"].tile([PART, DIM], bf16, tag=f"T1{g}")
                    T2 = p["T"].tile([PART, DIM], bf16, tag=f"T2{g}")
                    nc.vector.tensor_scalar(T1[:, :], P[:, k, :],
                                            wh[:, k : k + 1], None, Alu.mult)
                    nc.vector.tensor_scalar(T2[:, :], X[:, k, :],
                                            w2[:, k : k + 1], None, Alu.mult)
                    nc.vector.tensor_tensor(H[:, k, :], T1[:, :], T2[:, :],
                                            Alu.add)
                dst = state_t[g][PART * off : PART * (off + C)].rearrange(
                    "(c p) e -> p c e", p=PART)
                nc.sync.dma_start(dst, H[:, :, :])
                Hprev[g] = H
                prev_offs[g] = off
                continue

            # D = h_p - x
            pe_sub = SUBENG == "pe" and not is_perm
            if pe_sub:
                # D = I@P + (-I)@X on the PE, lands in PSUM fp32
                dsl = []
                for k in range(C):
                    ds = p["PS"].tile([PART, DIM], f32, tag=f"psq{g}")
                    dsl.append(ds)
                    nc.tensor.matmul(ds[:, :], ident_sb[:, :], P[:, k, :],
                                     start=True, stop=False)
                    nc.tensor.matmul(ds[:, :], negident_sb[:, :], X[:, k, :],
                                     start=False, stop=True)
                dk = lambda k: dsl[k][:, :]
            else:
                D = p["D"].tile([PART, C, DIM], bf16, tag=f"D{g}")
                if is_perm:
                    for k in range(C):
                        pick("sub", k).tensor_tensor(
                            D[:, k, :], par(k), X[:, k, :], Alu.subtract)
                else:
                    pick("sub", 0).tensor_tensor(D[:, :, :], P[:, :, :],
                                                 X[:, :, :], Alu.subtract)
                dk = lambda k: D[:, k, :]
            # z = <x, D> = <h_p, x> - <x, x>   (per chunk, fused mul+sum)
            for k in range(C):
                deng = pick("dot", k)
                if DOT4X and deng is nc.vector and not pe_sub:
                    M = p["T"].tile([PART, DIM], bf16, tag=f"M{g}")
                    deng.tensor_tensor(M[:, :], X[:, k, :], dk(k), Alu.mult)
                    deng.tensor_scalar(jt[deng][:, :], M[:, :], 1.0, 0.0,
                                       Alu.mult, Alu.add,
                                       accum_out=dp[:, k : k + 1])
                else:
                    deng.scalar_tensor_tensor(
                        jt[deng][:, :], X[:, k, :], 0.0, dk(k),
                        Alu.bypass, Alu.mult,
                        accum_out=dp[:, k : k + 1])
            # w = sigmoid(z) = alpha/(alpha+beta)
            nc.scalar.activation(wh[:, :], dp[:, :], Act.Sigmoid)
            # h = w*D + x
            if BLENDENG.startswith("act"):
                T = p["T"].tile([PART, C, DIM], bf16, tag=f"T{g}")
                for k in range(C):
                    nc.scalar.activation(T[:, k, :], dk(k), Act.Copy,
                                         scale=wh[:, k : k + 1])
                    aeng = nc.vector
                    if BLENDENG.startswith("actpool1of") and \
                            k % int(BLENDENG[10:]) == 0:
                        aeng = nc.gpsimd
                    aeng.tensor_tensor(H[:, k, :], T[:, k, :], X[:, k, :],
                                       Alu.add)
            else:
                for k in range(C):
                    beng = pick("blend", k)
                    if BLEND4X and beng is nc.vector and not pe_sub:
                        T = p["T"].tile([PART, DIM], bf16, tag=f"Tb{g}")
                        beng.tensor_scalar(T[:, :], dk(k), wh[:, k : k + 1],
                                           None, Alu.mult)
                        beng.tensor_tensor(H[:, k, :], T[:, :], X[:, k, :],
                                           Alu.add)
                    else:
                        beng.scalar_tensor_tensor(
                            H[:, k, :], dk(k), wh[:, k : k + 1], X[:, k, :],
                            Alu.mult, Alu.add)

            dst = state_t[g][PART * off : PART * (off + C)].rearrange(
                "(c p) e -> p c e", p=PART)
            nc.sync.dma_start(dst, H[:, :, :])
            Hprev[g] = H
            prev_offs[g] = off

        stack.close()

    nc.compile()
    return nc


def kernel(tree_embedding, node_connection, node_mask=None):
    import sys
    if "/opt/trn_rl_repo" not in sys.path:
        sys.path.insert(0, "/opt/trn_rl_repo")
    from concourse.bass_utils import run_bass_kernel_spmd

    emb = np.asarray(tree_embedding, dtype=np.float32)
    emb_bf = emb.astype(BF16)
    conn = np.asarray(node_connection).astype(np.int32)
    B, N, D = emb.shape
    assert D == DIM and B == N_CORES * STREAMS * TREES_PER_STREAM

    L, Cls, sched = _build_schedule(conn)
    permlevs = [sched[0][f"permlevs{g}"] for g in range(STREAMS)]
    nc = _build_program(L, Cls, permlevs)

    in_maps = []
    for c in range(N_CORES):
        m = {}
        for g in range(STREAMS):
            embS = _presort_emb(emb_bf, sched, c, g, Cls)
            m[f"embS{g}"] = embS
            m[f"pidx{g}"] = sched[c][f"pidx{g}"]
            if DYN:
                m[f"cnt{g}"] = sched[c][f"cnt{g}"]
            if sched[c][f"perm{g}"].shape[1]:
                m[f"perm{g}"] = sched[c][f"perm{g}"]
            if BLEND3:
                m[f"nn{g}"] = _presort_nn(embS)
        m["ident"] = np.eye(PART, dtype=BF16)
        in_maps.append(m)

    res = run_bass_kernel_spmd(nc, in_maps, list(range(N_CORES)))

    out = np.empty((B, N, DIM), np.float32)
    for c in range(N_CORES):
        for g in range(STREAMS):
            state = np.asarray(res.results[c][f"state{g}"]).astype(np.float32)
            posmat = sched[c][f"posmat{g}"]
            for t, b in enumerate(sched[c][f"trees{g}"]):
                out[b] = state[posmat[t]]
    return out
